# revision 20
# baseline (speedup 1.0000x reference)
"""Trainium2 Bass kernel for nn_Attention_7945689497706.

Distribution: data-parallel over batch, 2 batch elements per core, weights
replicated, no collectives.

v2 design (vs baseline):
  - Host folds (gamma+1) and dh^-0.5 into bf16 weights; x shipped bf16
    (halves input DMA, no on-chip weight prep).
  - Keys ordered per head as [pixels 0:1024 | mem(4)]: pixel chunks align
    with the vproj outputs, and mem_kv forms a tiny 9th chunk packed
    4-pairs-per-psum via col tiling; its K/V land via DMA (V is a host
    constant shared by both batches).
  - sim matmuls of a head PAIR run concurrently on disjoint PE row groups
    (even head rows 0:64, odd head rows 64:128) -> 2x sim throughput.
  - exp at [128, 1024] grain covering both heads of the pair.
  - softmax denominators via ones-column of v; per (pair, h2) a single K=2
    selector matmul broadcasts both heads' denominators, DVE reciprocal +
    psum-direct muls assemble attn.
  - Flat loop over 8 (batch, pair) units; qkv proj of batch 1 and out proj
    of batch 0 are spread as PE filler inside the loop.
"""

import numpy as np

import concourse.bass as bass
import concourse.mybir as mybir
import concourse.tile as tile
from concourse import bacc
from concourse.bass_utils import run_bass_kernel_spmd

F32 = mybir.dt.float32
F32R = mybir.dt.float32r
BF16 = mybir.dt.bfloat16
AF = mybir.ActivationFunctionType

NCORES = 8
B = 16
C = 512
N = 1024          # pixels = 32*32
HEADS = 8
DH = 64
NMEM = 4
PB = B // NCORES  # batch elements per core
CT = C // 128     # channel partition-tiles
NPAIR = 4         # head pairs
VW = DH + 1       # per head [v | ones]
KC = 1028         # keys per head: 1024 px + 4 mem


def _build():
    nc = bacc.Bacc()
    x_ext = nc.declare_dram_parameter("x", [PB, C, N], BF16, isOutput=False)
    wqkv_ext = nc.declare_dram_parameter("wqkv", [C, 3 * C], BF16, isOutput=False)
    wot_ext = nc.declare_dram_parameter("wot", [C, C], BF16, isOutput=False)
    memk_ext = nc.declare_dram_parameter("memk", [128, HEADS, NMEM], BF16, isOutput=False)
    memv9_ext = nc.declare_dram_parameter("memv9", [128, HEADS * VW], BF16, isOutput=False)
    out_ext = nc.declare_dram_parameter("out", [PB, C, N], F32, isOutput=True)

    with tile.TileContext(nc) as tc:
        with (
            tc.tile_pool(name="const", bufs=1) as const,
            tc.tile_pool(name="data", bufs=2) as data,
            tc.tile_pool(name="pp", bufs=6) as pp,
            tc.tile_pool(name="rp", bufs=2) as rp,
            tc.tile_pool(name="dvp", bufs=2) as dvp,
            tc.tile_pool(name="obp", bufs=2) as obp,
            tc.tile_pool(name="sim_ps", bufs=2, space="PSUM") as sim_ps,
            tc.tile_pool(name="av_ps", bufs=1, space="PSUM") as av_ps,
            tc.tile_pool(name="proj_ps", bufs=2, space="PSUM") as proj_ps,
        ):
            # ---------------- constants / big tiles ----------------
            wqkv = const.tile([128, CT, 3 * C], BF16, tag="wqkv")
            wo = const.tile([128, CT, C], BF16, tag="wo")
            ones128 = const.tile([128, 128], BF16, tag="ones128")
            ones1 = const.tile([128, 64], F32R, tag="ones1")

            def btiles(nm, shape, dtype):
                return [const.tile(shape, dtype, tag=f"{nm}{b}", name=f"{nm}{b}")
                        for b in range(PB)]

            xbs = btiles("xb", [128, CT, N], BF16)
            xns = btiles("xn", [128, CT, N], BF16)
            qTs = btiles("qT", [128, NPAIR, N], BF16)
            kTps = btiles("kTp", [128, HEADS, KC], BF16)
            vexts = btiles("vext", [128, 8, HEADS * VW], BF16)
            vext9 = const.tile([128, HEADS * VW], BF16, tag="v9", name="v9")
            p9s = btiles("p9", [128, 2, N], BF16)
            attns = btiles("attn", [128, CT, N], BF16)
            snorms = btiles("snorm", [128, N], F32)

            # ---------------- DMA plan ----------------
            # Few, large transfers (descriptor issue costs ~600ns of engine
            # time each). x on the sync queue; weights on the gpsimd queue so
            # the scalar engine stays free for activations.
            for t in range(CT):
                nc.sync.dma_start(out=xbs[0][:, t, :], in_=x_ext[0, t * 128:(t + 1) * 128, :])
            for t in range(CT):
                nc.gpsimd.dma_start(out=wqkv[:, t, :],
                                    in_=wqkv_ext[t * 128:(t + 1) * 128, :])
            for t in range(CT):
                nc.sync.dma_start(out=xbs[1][:, t, :], in_=x_ext[1, t * 128:(t + 1) * 128, :])
            for t in range(CT):
                nc.gpsimd.dma_start(out=wo[:, t, :], in_=wot_ext[t * 128:(t + 1) * 128, :])
            # mem K at kTp cols 1024:1028 (disjoint from kchain writes);
            # mem V (with its ones column, pair-p rows at 32p) is a host
            # constant shared by both batches.
            for b in range(PB):
                nc.gpsimd.dma_start(out=kTps[b][:, :, 1024:1028], in_=memk_ext[:, :, :])
            nc.gpsimd.dma_start(out=vext9, in_=memv9_ext[:, :])

            nc.vector.memset(ones128, 1.0)
            nc.vector.memset(ones1.bitcast(F32), 1.0)
            # ones columns of vext, full-partition memsets (chunk-7 mem rows are
            # later overwritten by the memv DMA, which also carries 1.0 there)
            for b in range(PB):
                oc = vexts[b][:, :, :].rearrange("p j (h c) -> p j h c", c=VW)[:, :, :, DH:DH + 1]
                nc.gpsimd.memset(oc, 1.0)

            # ---------------- norm ----------------
            def norm(b):
                xb = xbs[b]
                xsq = data.tile([128, CT, N], BF16, tag="xsq", bufs=1)
                for t in range(CT):
                    nc.vector.tensor_mul(out=xsq[:, t, :], in0=xb[:, t, :], in1=xb[:, t, :])
                ss = sim_ps.tile([128, N], F32, tag="sim")
                for h2 in range(2):
                    for t in range(CT):
                        nc.tensor.matmul(ss[:, h2 * 512:(h2 + 1) * 512], ones128,
                                         xsq[:, t, h2 * 512:(h2 + 1) * 512],
                                         start=(t == 0), stop=(t == CT - 1))
                sroot = data.tile([128, N], F32, tag="sroot")
                nc.scalar.activation(out=sroot, in_=ss, func=AF.Sqrt, scale=1.0 / C)
                nc.vector.reciprocal_approx_fast(out=snorms[b], in_=sroot)
                for t in range(CT):
                    nc.vector.tensor_mul(out=xns[b][:, t, :], in0=xb[:, t, :], in1=snorms[b])

            # ---------------- projection chains ----------------
            def qchain(b, p, h2):
                ps = proj_ps.tile([128, 512], F32, tag="proj")
                for t in range(CT):
                    nc.tensor.matmul(ps, wqkv[:, t, 128 * p:128 * (p + 1)],
                                     xns[b][:, t, h2 * 512:(h2 + 1) * 512],
                                     start=(t == 0), stop=(t == CT - 1))
                nc.vector.tensor_copy(out=qTs[b][:, p, h2 * 512:(h2 + 1) * 512], in_=ps)

            def kchain(b, p, h2):
                ps = proj_ps.tile([128, 512], F32, tag="proj")
                for t in range(CT):
                    nc.tensor.matmul(ps, wqkv[:, t, C + 128 * p:C + 128 * (p + 1)],
                                     xns[b][:, t, h2 * 512:(h2 + 1) * 512],
                                     start=(t == 0), stop=(t == CT - 1))
                kTp = kTps[b]
                h0, h1 = 2 * p, 2 * p + 1
                c_lo, c_hi = h2 * 512, (h2 + 1) * 512
                nc.vector.tensor_copy(out=kTp[0:64, h0, c_lo:c_hi], in_=ps[0:64, :])
                nc.vector.tensor_copy(out=kTp[64:128, h1, c_lo:c_hi], in_=ps[64:128, :])

            def vchain(b, ic):
                ps = proj_ps.tile([128, 512], F32, tag="proj")
                for t in range(CT):
                    nc.tensor.matmul(ps, xns[b][:, t, ic * 128:(ic + 1) * 128],
                                     wqkv[:, t, 2 * C:3 * C],
                                     start=(t == 0), stop=(t == CT - 1))
                ps_h = ps[:, :].rearrange("p (h c) -> p h c", c=DH)
                vdst = vexts[b][:, ic, :].rearrange("p (h c) -> p h c", c=VW)[:, :, 0:DH]
                nc.vector.tensor_copy(out=vdst, in_=ps_h)

            def ochain(b, mc, h2):
                ps = proj_ps.tile([128, 512], F32, tag="proj")
                for t in range(CT):
                    nc.tensor.matmul(ps, wo[:, t, mc * 128:(mc + 1) * 128],
                                     attns[b][:, t, h2 * 512:(h2 + 1) * 512],
                                     start=(t == 0), stop=(t == CT - 1))
                ob = obp.tile([128, 512], F32, tag="ob")
                nc.vector.tensor_copy(out=ob, in_=ps)
                nc.sync.dma_start(
                    out=out_ext[b, mc * 128:(mc + 1) * 128, h2 * 512:(h2 + 1) * 512],
                    in_=ob)

            # ---------------- chunk-8 (leftover 4 pixels), all pairs packed ----
            def sim9(b):
                kTp, qT = kTps[b], qTs[b]
                for h2 in range(2):
                    st9e = proj_ps.tile([128, 512], F32, tag="proj")
                    for p in range(NPAIR):
                        nc.tensor.matmul(st9e[32 * p:32 * p + NMEM, :],
                                         kTp[0:64, 2 * p, 1024:1028],
                                         qT[0:64, p, h2 * 512:(h2 + 1) * 512],
                                         start=True, stop=True,
                                         tile_position=(0, 32 * p))
                    nc.scalar.activation(out=p9s[b][:, h2, 0:512], in_=st9e, func=AF.Exp)
                    st9o = proj_ps.tile([128, 512], F32, tag="proj")
                    for p in range(NPAIR):
                        nc.tensor.matmul(st9o[32 * p:32 * p + NMEM, :],
                                         kTp[64:128, 2 * p + 1, 1024:1028],
                                         qT[64:128, p, h2 * 512:(h2 + 1) * 512],
                                         start=True, stop=True,
                                         tile_position=(64, 32 * p))
                    nc.scalar.activation(out=p9s[b][:, h2, 512:1024], in_=st9o, func=AF.Exp)

            # ---------------- one attention unit: (batch, pair, h2) ----------
            # Software-pipelined emission: sims/exps run 2 chunks ahead of the
            # av accumulations, and the av tail + normalization drain of unit
            # u-1 is emitted after unit u's first two sims, so the ACT exp
            # stream never waits on the av/drain dependency chain.
            pending_tail = [None]

            def attn_unit(b, p, h2, filler):
                kTp, qT, vext = kTps[b], qTs[b], vexts[b]
                h0, h1 = 2 * p, 2 * p + 1
                c0, c1 = h0 * VW, h1 * VW
                avE = av_ps.tile([128, 512], F32, tag="avE")
                avO = av_ps.tile([128, 512], F32, tag="avO")
                pts = {}

                def av(jc, stop):
                    nc.tensor.matmul(avE[0:VW, :], vext[:, jc, c0:c0 + VW],
                                     pts[jc][:, 0:512], start=(jc == 0), stop=stop)
                    nc.tensor.matmul(avO[0:VW, :], vext[:, jc, c1:c1 + VW],
                                     pts[jc][:, 512:1024], start=(jc == 0), stop=stop)

                for jc in range(8):
                    st = sim_ps.tile([128, N], F32, tag="sim")
                    nc.tensor.matmul(st[:, 0:512],
                                     kTp[0:64, h0, jc * 128:(jc + 1) * 128],
                                     qT[0:64, p, h2 * 512:(h2 + 1) * 512],
                                     start=True, stop=True)
                    nc.tensor.matmul(st[:, 512:1024],
                                     kTp[64:128, h1, jc * 128:(jc + 1) * 128],
                                     qT[64:128, p, h2 * 512:(h2 + 1) * 512],
                                     start=True, stop=True)
                    pt = pp.tile([128, N], BF16, tag="p")
                    nc.scalar.activation(out=pt, in_=st, func=AF.Exp)
                    pts[jc] = pt
                    if jc == 1 and pending_tail[0] is not None:
                        pending_tail[0]()
                        pending_tail[0] = None
                    if jc >= 2:
                        av(jc - 2, stop=False)
                    if filler is not None:
                        filler(b, p, h2, jc)

                def tail():
                    av(6, stop=False)
                    av(7, stop=False)
                    nc.tensor.matmul(avE[0:VW, :],
                                     vext9[32 * p:32 * p + NMEM, c0:c0 + VW],
                                     p9s[b][32 * p:32 * p + NMEM, h2, 0:512],
                                     start=False, stop=True,
                                     tile_position=(32 * p, 0))
                    nc.tensor.matmul(avO[0:VW, :],
                                     vext9[32 * p:32 * p + NMEM, c1:c1 + VW],
                                     p9s[b][32 * p:32 * p + NMEM, h2, 512:1024],
                                     start=False, stop=True,
                                     tile_position=(32 * p, 0))
                    dv = dvp.tile([128, 1024], F32R, tag="dv")
                    nc.vector.tensor_copy(out=dv[64:65, 0:512], in_=avE[64:65, :])
                    nc.vector.tensor_copy(out=dv[64:65, 512:1024], in_=avO[64:65, :])
                    bcs = sim_ps.tile([128, N], F32, tag="sim")
                    nc.tensor.matmul(bcs[0:64, 0:512], ones1[64:65, :], dv[64:65, 0:512],
                                     start=True, stop=True)
                    nc.tensor.matmul(bcs[0:64, 512:1024], ones1[64:65, :], dv[64:65, 512:1024],
                                     start=True, stop=True)
                    rcpE = rp.tile([64, 512], F32, tag="rcpE")
                    nc.vector.reciprocal_approx_fast(out=rcpE, in_=bcs[0:64, 0:512])
                    rcpO = rp.tile([64, 512], F32, tag="rcpO")
                    nc.vector.reciprocal_approx_fast(out=rcpO, in_=bcs[0:64, 512:1024])
                    nc.vector.tensor_mul(
                        out=attns[b][0:64, p, h2 * 512:(h2 + 1) * 512],
                        in0=avE[0:64, :], in1=rcpE)
                    nc.vector.tensor_mul(
                        out=attns[b][64:128, p, h2 * 512:(h2 + 1) * 512],
                        in0=avO[0:64, :], in1=rcpO)

                pending_tail[0] = tail

            # ---------------- schedule ----------------
            # Pre-loop: just enough for the first unit and the mem-sims
            # (which only need the q chains + the mem-K DMA). Remaining
            # batch-0 chains feed the early filler queue so the proj ring
            # never holds far-from-ready tiles ahead of urgent ones.
            norm(0)
            norm(1)
            qchain(0, 0, 0)
            qchain(0, 0, 1)
            kchain(0, 0, 0)
            kchain(0, 0, 1)
            vchain(0, 0)
            qchain(0, 1, 0)
            qchain(0, 1, 1)
            vchain(0, 1)
            qchain(0, 2, 0)
            qchain(0, 2, 1)
            vchain(0, 2)
            qchain(0, 3, 0)
            qchain(0, 3, 1)
            sim9(0)

            fills_e = []
            for it in [lambda: kchain(0, 1, 0), lambda: kchain(0, 1, 1),
                       lambda: vchain(0, 3), lambda: vchain(0, 4),
                       lambda: vchain(0, 5), lambda: vchain(0, 6),
                       lambda: vchain(0, 7),
                       lambda: kchain(0, 2, 0), lambda: kchain(0, 2, 1),
                       lambda: kchain(0, 3, 0), lambda: kchain(0, 3, 1)]:
                fills_e.append(it)
            fills_a = []
            for p in range(NPAIR):
                for h2 in range(2):
                    fills_a.append(lambda p=p, h2=h2: kchain(1, p, h2))
                    fills_a.append(lambda p=p, h2=h2: qchain(1, p, h2))
            fills_a.append(lambda: sim9(1))
            for ic in range(8):
                fills_a.append(lambda ic=ic: vchain(1, ic))
            fills_b = []
            for mc in range(CT):
                for h2 in range(2):
                    fills_b.append(lambda mc=mc, h2=h2: ochain(0, mc, h2))
            fcount = [0, 0, 0]

            def filler(b, p, h2, jc):
                unit = (b * NPAIR + p) * 16 + h2 * 8 + jc
                want_e = min(len(fills_e), 2 * unit + 2)
                while fcount[2] < want_e:
                    fills_e[fcount[2]]()
                    fcount[2] += 1
                want_a = 0 if unit < 22 else min(len(fills_a), ((unit - 21) * 25) // 36)
                while fcount[0] < want_a:
                    fills_a[fcount[0]]()
                    fcount[0] += 1
                want_b = min(len(fills_b), max(0, ((unit - 60) * 8) // 48))
                while fcount[1] < want_b:
                    fills_b[fcount[1]]()
                    fcount[1] += 1

            for b in range(PB):
                for p in range(NPAIR):
                    for h2 in range(2):
                        attn_unit(b, p, h2, filler)
            pending_tail[0]()
            pending_tail[0] = None
            while fcount[2] < len(fills_e):
                fills_e[fcount[2]]()
                fcount[2] += 1
            while fcount[0] < len(fills_a):
                fills_a[fcount[0]]()
                fcount[0] += 1
            while fcount[1] < len(fills_b):
                fills_b[fcount[1]]()
                fcount[1] += 1
            for mc in range(CT):
                for h2 in range(2):
                    ochain(1, mc, h2)
    nc.compile()
    return nc


_NC_CACHE = []


def _prep_inputs(x, gamma, mem_kv, w_qkv, w_out):
    b, c, hh, ww = x.shape
    n = hh * ww
    xs = x.reshape(b, c, n)

    g1 = gamma + 1.0
    wq = w_qkv[0:C] * (DH ** -0.5)
    wkv = w_qkv[C:]
    wqkv_eff = np.concatenate([wq, wkv], axis=0) * g1[None, :]
    wqkvt = np.ascontiguousarray(wqkv_eff.T)       # [c, 3c]
    wot = np.ascontiguousarray(w_out.T)            # [c, c]

    # memk: [128, heads, 4] - even head rows 0:64, odd head rows 64:128
    memk = np.zeros((128, HEADS, NMEM), np.float32)
    for h in range(HEADS):
        r0 = 64 * (h % 2)
        memk[r0:r0 + DH, h] = mem_kv[0, h].T
    # memv9: [128, heads*(dh+1)] - pair p's mem V at rows 32p:32p+4,
    # v then the ones column
    memv = np.zeros((128, HEADS * VW), np.float32)
    for h in range(HEADS):
        r0 = 32 * (h // 2)
        memv[r0:r0 + NMEM, h * VW:h * VW + DH] = mem_kv[1, h]
        memv[r0:r0 + NMEM, h * VW + DH] = 1.0

    import jax.numpy as jnp

    def bf(a):
        return np.asarray(jnp.asarray(a, dtype=jnp.bfloat16))

    return xs, bf(xs), bf(wqkvt), bf(wot), bf(memk), bf(memv)


def kernel(x, gamma, mem_kv, w_qkv, w_out, _trace=False):
    x = np.asarray(x, dtype=np.float32)
    gamma = np.asarray(gamma, dtype=np.float32)
    mem_kv = np.asarray(mem_kv, dtype=np.float32)
    w_qkv = np.asarray(w_qkv, dtype=np.float32)
    w_out = np.asarray(w_out, dtype=np.float32)

    b, c, hh, ww = x.shape
    xs, xbf, wqkvt, wot, memk, memv = _prep_inputs(x, gamma, mem_kv, w_qkv, w_out)

    if not _NC_CACHE:
        _NC_CACHE.append(_build())
    nc = _NC_CACHE[0]

    in_maps = []
    for core in range(NCORES):
        in_maps.append({
            "x": np.ascontiguousarray(xbf[core * PB:(core + 1) * PB]),
            "wqkv": wqkvt,
            "wot": wot,
            "memk": memk,
            "memv9": memv,
        })
    res = run_bass_kernel_spmd(nc, in_maps, core_ids=list(range(NCORES)), trace=_trace)
    out = np.concatenate([res.results[core]["out"] for core in range(NCORES)], axis=0)
    kernel.last_result = res
    return out.reshape(b, c, hh, ww)


# revision 21
# speedup vs baseline: 1.0926x; 1.0926x over previous
"""Trainium2 Bass kernel for nn_Attention_7945689497706.

Distribution: data-parallel over batch, 2 batch elements per core, weights
replicated, no collectives.

v2 design (vs baseline):
  - Host folds (gamma+1) and dh^-0.5 into bf16 weights; x shipped bf16
    (halves input DMA, no on-chip weight prep).
  - Keys ordered per head as [pixels 0:1024 | mem(4)]: pixel chunks align
    with the vproj outputs, and mem_kv forms a tiny 9th chunk packed
    4-pairs-per-psum via col tiling; its K/V land via DMA (V is a host
    constant shared by both batches).
  - sim matmuls of a head PAIR run concurrently on disjoint PE row groups
    (even head rows 0:64, odd head rows 64:128) -> 2x sim throughput.
  - exp at [128, 1024] grain covering both heads of the pair.
  - softmax denominators via ones-column of v; per (pair, h2) a single K=2
    selector matmul broadcasts both heads' denominators, DVE reciprocal +
    psum-direct muls assemble attn.
  - Flat loop over 8 (batch, pair) units; qkv proj of batch 1 and out proj
    of batch 0 are spread as PE filler inside the loop.
"""

import numpy as np

import concourse.bass as bass
import concourse.mybir as mybir
import concourse.tile as tile
from concourse import bacc
from concourse.bass_utils import run_bass_kernel_spmd

F32 = mybir.dt.float32
F32R = mybir.dt.float32r
BF16 = mybir.dt.bfloat16
AF = mybir.ActivationFunctionType

NCORES = 8
B = 16
C = 512
N = 1024          # pixels = 32*32
HEADS = 8
DH = 64
NMEM = 4
PB = B // NCORES  # batch elements per core
CT = C // 128     # channel partition-tiles
NPAIR = 4         # head pairs
VW = DH + 1       # per head [v | ones]
KC = 1028         # keys per head: 1024 px + 4 mem


def _build():
    nc = bacc.Bacc()
    x_ext = nc.declare_dram_parameter("x", [PB, C, N], BF16, isOutput=False)
    wqkv_ext = nc.declare_dram_parameter("wqkv", [C, 3 * C], BF16, isOutput=False)
    wot_ext = nc.declare_dram_parameter("wot", [C, C], BF16, isOutput=False)
    memk_ext = nc.declare_dram_parameter("memk", [128, HEADS, NMEM], BF16, isOutput=False)
    memv9_ext = nc.declare_dram_parameter("memv9", [128, HEADS * VW], BF16, isOutput=False)
    out_ext = nc.declare_dram_parameter("out", [PB, C, N], F32, isOutput=True)

    with tile.TileContext(nc) as tc:
        with (
            tc.tile_pool(name="const", bufs=1) as const,
            tc.tile_pool(name="data", bufs=2) as data,
            tc.tile_pool(name="pp", bufs=6) as pp,
            tc.tile_pool(name="rp", bufs=2) as rp,
            tc.tile_pool(name="dvp", bufs=2) as dvp,
            tc.tile_pool(name="obp", bufs=2) as obp,
            tc.tile_pool(name="sim_ps", bufs=2, space="PSUM") as sim_ps,
            tc.tile_pool(name="av_ps", bufs=1, space="PSUM") as av_ps,
            tc.tile_pool(name="proj_ps", bufs=2, space="PSUM") as proj_ps,
        ):
            # ---------------- constants / big tiles ----------------
            wqkv = const.tile([128, CT, 3 * C], BF16, tag="wqkv")
            wo = const.tile([128, CT, C], BF16, tag="wo")
            ones128 = const.tile([128, 128], BF16, tag="ones128")
            ones1 = const.tile([128, 64], F32R, tag="ones1")

            def btiles(nm, shape, dtype):
                return [const.tile(shape, dtype, tag=f"{nm}{b}", name=f"{nm}{b}")
                        for b in range(PB)]

            xbs = btiles("xb", [128, CT, N], BF16)
            xns = btiles("xn", [128, CT, N], BF16)
            qTs = btiles("qT", [128, NPAIR, N], BF16)
            kTps = btiles("kTp", [128, HEADS, KC], BF16)
            vexts = btiles("vext", [128, 8, HEADS * VW], BF16)
            vext9 = const.tile([128, HEADS * VW], BF16, tag="v9", name="v9")
            p9s = btiles("p9", [128, 2, N], BF16)
            attns = btiles("attn", [128, CT, N], BF16)
            snorms = btiles("snorm", [128, N], F32)

            # ---------------- DMA plan ----------------
            # Few, large transfers (descriptor issue costs ~600ns of engine
            # time each). x on the sync queue; weights on the gpsimd queue so
            # the scalar engine stays free for activations.
            for t in range(CT):
                nc.sync.dma_start(out=xbs[0][:, t, :], in_=x_ext[0, t * 128:(t + 1) * 128, :])
            for t in range(CT):
                nc.gpsimd.dma_start(out=wqkv[:, t, :],
                                    in_=wqkv_ext[t * 128:(t + 1) * 128, :])
            for t in range(CT):
                nc.sync.dma_start(out=xbs[1][:, t, :], in_=x_ext[1, t * 128:(t + 1) * 128, :])
            for t in range(CT):
                nc.gpsimd.dma_start(out=wo[:, t, :], in_=wot_ext[t * 128:(t + 1) * 128, :])
            # mem K at kTp cols 1024:1028 (disjoint from kchain writes);
            # mem V (with its ones column, pair-p rows at 32p) is a host
            # constant shared by both batches.
            for b in range(PB):
                nc.gpsimd.dma_start(out=kTps[b][:, :, 1024:1028], in_=memk_ext[:, :, :])
            nc.gpsimd.dma_start(out=vext9, in_=memv9_ext[:, :])

            nc.vector.memset(ones128, 1.0)
            nc.vector.memset(ones1.bitcast(F32), 1.0)
            # ones columns of vext, full-partition memsets (chunk-7 mem rows are
            # later overwritten by the memv DMA, which also carries 1.0 there)
            for b in range(PB):
                oc = vexts[b][:, :, :].rearrange("p j (h c) -> p j h c", c=VW)[:, :, :, DH:DH + 1]
                nc.gpsimd.memset(oc, 1.0)

            # ---------------- norm ----------------
            def norm(b):
                xb = xbs[b]
                xsq = data.tile([128, CT, N], BF16, tag="xsq", bufs=1)
                for t in range(CT):
                    nc.vector.tensor_mul(out=xsq[:, t, :], in0=xb[:, t, :], in1=xb[:, t, :])
                ss = sim_ps.tile([128, N], F32, tag="sim")
                for h2 in range(2):
                    for t in range(CT):
                        nc.tensor.matmul(ss[:, h2 * 512:(h2 + 1) * 512], ones128,
                                         xsq[:, t, h2 * 512:(h2 + 1) * 512],
                                         start=(t == 0), stop=(t == CT - 1))
                sroot = data.tile([128, N], F32, tag="sroot")
                nc.scalar.activation(out=sroot, in_=ss, func=AF.Sqrt, scale=1.0 / C)
                nc.vector.reciprocal_approx_fast(out=snorms[b], in_=sroot)
                for t in range(CT):
                    nc.vector.tensor_mul(out=xns[b][:, t, :], in0=xb[:, t, :], in1=snorms[b])

            # ---------------- projection chains ----------------
            def qchain(b, p, h2):
                ps = proj_ps.tile([128, 512], F32, tag="proj")
                for t in range(CT):
                    nc.tensor.matmul(ps, wqkv[:, t, 128 * p:128 * (p + 1)],
                                     xns[b][:, t, h2 * 512:(h2 + 1) * 512],
                                     start=(t == 0), stop=(t == CT - 1))
                nc.vector.tensor_copy(out=qTs[b][:, p, h2 * 512:(h2 + 1) * 512], in_=ps)

            def kchain(b, p, h2):
                ps = proj_ps.tile([128, 512], F32, tag="proj")
                for t in range(CT):
                    nc.tensor.matmul(ps, wqkv[:, t, C + 128 * p:C + 128 * (p + 1)],
                                     xns[b][:, t, h2 * 512:(h2 + 1) * 512],
                                     start=(t == 0), stop=(t == CT - 1))
                kTp = kTps[b]
                h0, h1 = 2 * p, 2 * p + 1
                c_lo, c_hi = h2 * 512, (h2 + 1) * 512
                nc.vector.tensor_copy(out=kTp[0:64, h0, c_lo:c_hi], in_=ps[0:64, :])
                nc.vector.tensor_copy(out=kTp[64:128, h1, c_lo:c_hi], in_=ps[64:128, :])

            def vchain(b, ic):
                ps = proj_ps.tile([128, 512], F32, tag="proj")
                for t in range(CT):
                    nc.tensor.matmul(ps, xns[b][:, t, ic * 128:(ic + 1) * 128],
                                     wqkv[:, t, 2 * C:3 * C],
                                     start=(t == 0), stop=(t == CT - 1))
                ps_h = ps[:, :].rearrange("p (h c) -> p h c", c=DH)
                vdst = vexts[b][:, ic, :].rearrange("p (h c) -> p h c", c=VW)[:, :, 0:DH]
                nc.vector.tensor_copy(out=vdst, in_=ps_h)

            def ochain(b, mc, h2):
                ps = proj_ps.tile([128, 512], F32, tag="proj")
                for t in range(CT):
                    nc.tensor.matmul(ps, wo[:, t, mc * 128:(mc + 1) * 128],
                                     attns[b][:, t, h2 * 512:(h2 + 1) * 512],
                                     start=(t == 0), stop=(t == CT - 1))
                ob = obp.tile([128, 512], F32, tag="ob")
                nc.vector.tensor_copy(out=ob, in_=ps)
                nc.sync.dma_start(
                    out=out_ext[b, mc * 128:(mc + 1) * 128, h2 * 512:(h2 + 1) * 512],
                    in_=ob)

            # ---------------- chunk-8 (leftover 4 pixels), all pairs packed ----
            def sim9(b):
                kTp, qT = kTps[b], qTs[b]
                for h2 in range(2):
                    st9e = proj_ps.tile([128, 512], F32, tag="proj")
                    for p in range(NPAIR):
                        nc.tensor.matmul(st9e[32 * p:32 * p + NMEM, :],
                                         kTp[0:64, 2 * p, 1024:1028],
                                         qT[0:64, p, h2 * 512:(h2 + 1) * 512],
                                         start=True, stop=True,
                                         tile_position=(0, 32 * p))
                    nc.scalar.activation(out=p9s[b][:, h2, 0:512], in_=st9e, func=AF.Exp)
                    st9o = proj_ps.tile([128, 512], F32, tag="proj")
                    for p in range(NPAIR):
                        nc.tensor.matmul(st9o[32 * p:32 * p + NMEM, :],
                                         kTp[64:128, 2 * p + 1, 1024:1028],
                                         qT[64:128, p, h2 * 512:(h2 + 1) * 512],
                                         start=True, stop=True,
                                         tile_position=(64, 32 * p))
                    nc.scalar.activation(out=p9s[b][:, h2, 512:1024], in_=st9o, func=AF.Exp)

            # ---------------- one attention unit: (batch, pair, h2) ----------
            # Software-pipelined emission: sims/exps run 2 chunks ahead of the
            # av accumulations, and the av tail + normalization drain of unit
            # u-1 is emitted after unit u's first two sims, so the ACT exp
            # stream never waits on the av/drain dependency chain.
            pending_tail = [None]

            def attn_unit(b, p, h2, filler):
                kTp, qT, vext = kTps[b], qTs[b], vexts[b]
                h0, h1 = 2 * p, 2 * p + 1
                c0, c1 = h0 * VW, h1 * VW
                avE = av_ps.tile([128, 512], F32, tag="avE")
                avO = av_ps.tile([128, 512], F32, tag="avO")
                pts = {}

                def av(jc, stop):
                    nc.tensor.matmul(avE[0:VW, :], vext[:, jc, c0:c0 + VW],
                                     pts[jc][:, 0:512], start=(jc == 0), stop=stop)
                    nc.tensor.matmul(avO[0:VW, :], vext[:, jc, c1:c1 + VW],
                                     pts[jc][:, 512:1024], start=(jc == 0), stop=stop)

                for jc in range(8):
                    st = sim_ps.tile([128, N], F32, tag="sim")
                    nc.tensor.matmul(st[:, 0:512],
                                     kTp[0:64, h0, jc * 128:(jc + 1) * 128],
                                     qT[0:64, p, h2 * 512:(h2 + 1) * 512],
                                     start=True, stop=True)
                    nc.tensor.matmul(st[:, 512:1024],
                                     kTp[64:128, h1, jc * 128:(jc + 1) * 128],
                                     qT[64:128, p, h2 * 512:(h2 + 1) * 512],
                                     start=True, stop=True)
                    pt = pp.tile([128, N], BF16, tag="p")
                    nc.scalar.activation(out=pt, in_=st, func=AF.Exp)
                    pts[jc] = pt
                    if jc == 1 and pending_tail[0] is not None:
                        pending_tail[0]()
                        pending_tail[0] = None
                    if jc >= 2:
                        av(jc - 2, stop=False)
                    if filler is not None:
                        filler(b, p, h2, jc)

                def tail():
                  with tc.high_priority(offset=-64):
                    av(6, stop=False)
                    av(7, stop=False)
                    nc.tensor.matmul(avE[0:VW, :],
                                     vext9[32 * p:32 * p + NMEM, c0:c0 + VW],
                                     p9s[b][32 * p:32 * p + NMEM, h2, 0:512],
                                     start=False, stop=True,
                                     tile_position=(32 * p, 0))
                    nc.tensor.matmul(avO[0:VW, :],
                                     vext9[32 * p:32 * p + NMEM, c1:c1 + VW],
                                     p9s[b][32 * p:32 * p + NMEM, h2, 512:1024],
                                     start=False, stop=True,
                                     tile_position=(32 * p, 0))
                    dv = dvp.tile([128, 1024], F32R, tag="dv")
                    nc.vector.tensor_copy(out=dv[64:65, 0:512], in_=avE[64:65, :])
                    nc.vector.tensor_copy(out=dv[64:65, 512:1024], in_=avO[64:65, :])
                    bcpE = proj_ps.tile([128, 512], F32, tag="proj")
                    nc.tensor.matmul(bcpE[0:64, :], ones1[64:65, :], dv[64:65, 0:512],
                                     start=True, stop=True)
                    bcpO = proj_ps.tile([128, 512], F32, tag="proj")
                    nc.tensor.matmul(bcpO[0:64, :], ones1[64:65, :], dv[64:65, 512:1024],
                                     start=True, stop=True)
                    rcpE = rp.tile([64, 512], F32, tag="rcpE")
                    nc.vector.reciprocal_approx_fast(out=rcpE, in_=bcpE[0:64, :])
                    rcpO = rp.tile([64, 512], F32, tag="rcpO")
                    nc.vector.reciprocal_approx_fast(out=rcpO, in_=bcpO[0:64, :])
                    nc.vector.tensor_mul(
                        out=attns[b][0:64, p, h2 * 512:(h2 + 1) * 512],
                        in0=avE[0:64, :], in1=rcpE)
                    nc.vector.tensor_mul(
                        out=attns[b][64:128, p, h2 * 512:(h2 + 1) * 512],
                        in0=avO[0:64, :], in1=rcpO)

                pending_tail[0] = tail

            # ---------------- schedule ----------------
            # Pre-loop: just enough for the first unit and the mem-sims
            # (which only need the q chains + the mem-K DMA). Remaining
            # batch-0 chains feed the early filler queue so the proj ring
            # never holds far-from-ready tiles ahead of urgent ones.
            norm(0)
            norm(1)
            qchain(0, 0, 0)
            qchain(0, 0, 1)
            kchain(0, 0, 0)
            kchain(0, 0, 1)
            vchain(0, 0)
            qchain(0, 1, 0)
            qchain(0, 1, 1)
            vchain(0, 1)
            qchain(0, 2, 0)
            qchain(0, 2, 1)
            vchain(0, 2)
            qchain(0, 3, 0)
            qchain(0, 3, 1)
            sim9(0)
            vchain(0, 3)

            fills_e = []
            for it in [lambda: vchain(0, 4), lambda: vchain(0, 5),
                       lambda: vchain(0, 6), lambda: vchain(0, 7),
                       lambda: kchain(0, 1, 0), lambda: kchain(0, 1, 1),
                       lambda: kchain(0, 2, 0), lambda: kchain(0, 2, 1),
                       lambda: kchain(0, 3, 0), lambda: kchain(0, 3, 1)]:
                fills_e.append(it)
            fills_a = []
            for p in range(NPAIR):
                for h2 in range(2):
                    fills_a.append(lambda p=p, h2=h2: kchain(1, p, h2))
                    fills_a.append(lambda p=p, h2=h2: qchain(1, p, h2))
            fills_a.append(lambda: sim9(1))
            for ic in range(8):
                fills_a.append(lambda ic=ic: vchain(1, ic))
            fills_b = []
            for mc in range(CT):
                for h2 in range(2):
                    fills_b.append(lambda mc=mc, h2=h2: ochain(0, mc, h2))
            fcount = [0, 0, 0]

            def filler(b, p, h2, jc):
                unit = (b * NPAIR + p) * 16 + h2 * 8 + jc
                want_e = min(len(fills_e), 2 * unit + 2)
                while fcount[2] < want_e:
                    fills_e[fcount[2]]()
                    fcount[2] += 1
                want_a = 0 if unit < 22 else min(len(fills_a), ((unit - 21) * 25) // 36)
                while fcount[0] < want_a:
                    fills_a[fcount[0]]()
                    fcount[0] += 1
                want_b = min(len(fills_b), max(0, ((unit - 63) * 8) // 37))
                while fcount[1] < want_b:
                    fills_b[fcount[1]]()
                    fcount[1] += 1

            for b in range(PB):
                for p in range(NPAIR):
                    for h2 in range(2):
                        attn_unit(b, p, h2, filler)
            pending_tail[0]()
            pending_tail[0] = None
            while fcount[2] < len(fills_e):
                fills_e[fcount[2]]()
                fcount[2] += 1
            while fcount[0] < len(fills_a):
                fills_a[fcount[0]]()
                fcount[0] += 1
            while fcount[1] < len(fills_b):
                fills_b[fcount[1]]()
                fcount[1] += 1
            for mc in range(CT):
                for h2 in range(2):
                    ochain(1, mc, h2)
    nc.compile()
    return nc


_NC_CACHE = []


def _prep_inputs(x, gamma, mem_kv, w_qkv, w_out):
    b, c, hh, ww = x.shape
    n = hh * ww
    xs = x.reshape(b, c, n)

    g1 = gamma + 1.0
    wq = w_qkv[0:C] * (DH ** -0.5)
    wkv = w_qkv[C:]
    wqkv_eff = np.concatenate([wq, wkv], axis=0) * g1[None, :]
    wqkvt = np.ascontiguousarray(wqkv_eff.T)       # [c, 3c]
    wot = np.ascontiguousarray(w_out.T)            # [c, c]

    # memk: [128, heads, 4] - even head rows 0:64, odd head rows 64:128
    memk = np.zeros((128, HEADS, NMEM), np.float32)
    for h in range(HEADS):
        r0 = 64 * (h % 2)
        memk[r0:r0 + DH, h] = mem_kv[0, h].T
    # memv9: [128, heads*(dh+1)] - pair p's mem V at rows 32p:32p+4,
    # v then the ones column
    memv = np.zeros((128, HEADS * VW), np.float32)
    for h in range(HEADS):
        r0 = 32 * (h // 2)
        memv[r0:r0 + NMEM, h * VW:h * VW + DH] = mem_kv[1, h]
        memv[r0:r0 + NMEM, h * VW + DH] = 1.0

    import jax.numpy as jnp

    def bf(a):
        return np.asarray(jnp.asarray(a, dtype=jnp.bfloat16))

    return xs, bf(xs), bf(wqkvt), bf(wot), bf(memk), bf(memv)


def kernel(x, gamma, mem_kv, w_qkv, w_out, _trace=False):
    x = np.asarray(x, dtype=np.float32)
    gamma = np.asarray(gamma, dtype=np.float32)
    mem_kv = np.asarray(mem_kv, dtype=np.float32)
    w_qkv = np.asarray(w_qkv, dtype=np.float32)
    w_out = np.asarray(w_out, dtype=np.float32)

    b, c, hh, ww = x.shape
    xs, xbf, wqkvt, wot, memk, memv = _prep_inputs(x, gamma, mem_kv, w_qkv, w_out)

    if not _NC_CACHE:
        _NC_CACHE.append(_build())
    nc = _NC_CACHE[0]

    in_maps = []
    for core in range(NCORES):
        in_maps.append({
            "x": np.ascontiguousarray(xbf[core * PB:(core + 1) * PB]),
            "wqkv": wqkvt,
            "wot": wot,
            "memk": memk,
            "memv9": memv,
        })
    res = run_bass_kernel_spmd(nc, in_maps, core_ids=list(range(NCORES)), trace=_trace)
    out = np.concatenate([res.results[core]["out"] for core in range(NCORES)], axis=0)
    kernel.last_result = res
    return out.reshape(b, c, hh, ww)
